# revision 9
# baseline (speedup 1.0000x reference)
"""Trainium2 Bass kernel for the FlowNet-style correlation layer.

Problem (hardcoded):
  x_1, x_2, p_1, p_2: [1, 64, 96, 96] f32;  img: [1, 1, 96, 96] f32
  x1 = concat(x_1, p_1) -> [1,128,96,96];  x2 = pad(concat(x_2,p_2), 20)
  out_vb[d, h, w]  = sum_c x1[c,h,w] * x2[c, h+dy, w+dx],  d = dy*41+dx
  out_img[d, h, w] = pad(img,20)[h+dy, w+dx]
  returns (out_vb [1,1681,96,96], out_img [1,1681,96,96])

Strategy: 8 cores tile the output plane 4x2 (24 h-rows x 48 w-cols each).
Within a core, 9 groups of 8h x 16w x1 pixels are packed as the 128-wide
stationary operand (hi*16+wj), so every PE column is live. The moving
operand is a 48-row x 64-col window of the core's zero-padded x2 slab; the
TensorEngine contracts over the 128 concat-channels, producing the group
Gram G[(hi,wj), (t,v)] = sum_c x1[c,hi,wj] * x2pad[c, hi+t.., wj+v..] in
fp32 PSUM. The correlation band is G[., (hi+dy, wj+dx)] -- every (dy,dx)
displacement of every packed pixel lands inside the 48x56 useful window
(redundancy only 1.63x vs the 2.6x of a full-row Gram). A single bf16
pass suffices for the 2e-2 gate (inputs are unit-normal; err ~3e-3).
PSUM is evacuated by Vector/Scalar/Pool copies that cast f32->f16 and
drop the 8 pad columns that round the matmul free size up to 512 (one
PSUM bank); the host extracts the diagonal band from the fp16 Gram with a
strided view while unsharding. out_img is a pure gather of the (host-
held) input image and is materialized host-side.
"""

import numpy as np

import concourse.bass as bass
import concourse.tile as tile
from concourse import bacc, mybir
from concourse.bass_types import AP
from concourse.bass_utils import run_bass_kernel_spmd

F32 = mybir.dt.float32
F16 = mybir.dt.float16
BF16 = mybir.dt.bfloat16

H = W = 96
C2 = 128            # concat channels
PAD = 20
D = 2 * PAD + 1     # 41 displacements per axis
NCORES = 8
CH, CW = 4, 2       # core grid (h x w)
HS = H // CH        # 24 output rows per core
WS = W // CW        # 48 output cols per core
HG, WG = 8, 16      # stationary packing: 8 h-rows x 16 w-cols = 128
NGH, NGW = HS // HG, WS // WG   # 3 x 3 groups per core
NG = NGH * NGW
TR = HG + D - 1     # 48 gram rows (t = hi + dy)
TV = WG + D - 1     # 56 useful gram cols (v = wj + dx)
TVP = 64            # padded to 64 so each matmul is exactly one PSUM bank
SR = HS + 2 * PAD   # 64 x2 slab rows per core
SC = WS + 2 * PAD + (TVP - TV)  # 96 x2 slab cols (incl. 8 junk-read pad)
GSZ = TR * TV       # 2688 gram elems per partition per group


def _build_nc():
    nc = bacc.Bacc("TRN2", target_bir_lowering=False, debug=False,
                   num_devices=NCORES)

    x1p = nc.declare_dram_parameter("x1p", [C2, NG * 128], BF16,
                                    isOutput=False)
    x2s = nc.declare_dram_parameter("x2s", [C2, SR, SC], BF16, isOutput=False)
    gram = nc.declare_dram_parameter("gram", [NG, 128, GSZ], F16,
                                     isOutput=True)

    with tile.TileContext(nc) as tc:
        with (
            tc.tile_pool(name="inp", bufs=1) as pin,
            tc.tile_pool(name="stage", bufs=3) as pst,
            tc.tile_pool(name="psum", bufs=4, space="PSUM") as pps,
        ):
            x1p_sb = pin.tile([C2, NG * 128], BF16)
            nc.sync.dma_start(x1p_sb[:], x1p[:])
            x2s_sb = pin.tile([C2, SR, SC], BF16)
            # rows [0, 48) feed the first (gh=0) groups; stream them first,
            # split across both DMA queues, so matmuls start ASAP while
            # rows [48, 64) trail in.
            nc.scalar.dma_start(x2s_sb[:, 0:TR // 2, :], x2s[:, 0:TR // 2, :])
            nc.sync.dma_start(x2s_sb[:, TR // 2:TR, :], x2s[:, TR // 2:TR, :])
            nc.scalar.dma_start(x2s_sb[:, TR:, :], x2s[:, TR:, :])

            def _copy(k, dst, src):
                if k % 2 == 0:
                    nc.vector.tensor_copy(dst, src)
                else:
                    nc.scalar.copy(dst, src)

            for g in range(NG):
                gh, gw = divmod(g, NGW)
                stat = x1p_sb[:, g * 128:(g + 1) * 128]
                stage = pst.tile([C2, 3, 2, HG, TV], F16)
                for third in range(3):
                    ps = pps.tile([C2, 2, HG, TVP], F32)  # 2 PSUM banks
                    for m in range(2):
                        r0 = HG * gh + HG * (third * 2 + m)
                        nc.tensor.matmul(
                            ps[:, m], stat,
                            x2s_sb[:, r0:r0 + HG, WG * gw:WG * gw + TVP],
                            start=True, stop=True)
                    _copy(g * 3 + third, stage[:, third], ps[:, :, :, 0:TV])
                dst = AP(tensor=gram[:].tensor,
                         offset=g * 128 * GSZ,
                         ap=[[GSZ, 128], [1, GSZ]])
                eng = nc.sync if g % 2 == 0 else nc.scalar
                eng.dma_start(dst, stage[:])

    nc.compile()
    return nc


_NC_CACHE = None


def _get_nc():
    global _NC_CACHE
    if _NC_CACHE is None:
        _NC_CACHE = _build_nc()
    return _NC_CACHE


def _prep_in_maps(x_1, x_2, img, p_1, p_2):
    import ml_dtypes
    bf = ml_dtypes.bfloat16

    x1f = np.concatenate([x_1[0], p_1[0]], axis=0).astype(bf)
    x2f = np.concatenate([x_2[0], p_2[0]], axis=0).astype(bf)
    x2pad = np.zeros((C2, H + 2 * PAD, W + 2 * PAD + (TVP - TV)), bf)
    x2pad[:, PAD:PAD + H, PAD:PAD + W] = x2f

    in_maps = []
    for ci in range(CH):
        for cj in range(CW):
            h0, w0 = ci * HS, cj * WS
            x1c = x1f[:, h0:h0 + HS, w0:w0 + WS]
            x1pk = (x1c.reshape(C2, NGH, HG, NGW, WG)
                    .transpose(0, 1, 3, 2, 4).reshape(C2, NG * 128))
            in_maps.append({
                "x1p": np.ascontiguousarray(x1pk),
                "x2s": np.ascontiguousarray(
                    x2pad[:, h0:h0 + SR, w0:w0 + SC]),
            })
    return in_maps


def _postprocess(results, img):
    out_vb = np.empty((1, D * D, H, W), np.float32)
    k = 0
    for ci in range(CH):
        for cj in range(CW):
            g16 = np.asarray(results[k]["gram"])   # [9, 128, 2688] f16
            k += 1
            A = g16.reshape(NGH, NGW, HG, WG, TR, TV)
            s = A.strides
            v = np.lib.stride_tricks.as_strided(
                A, shape=(D, D, NGH, HG, NGW, WG),
                strides=(s[4], s[5], s[0], s[2] + s[4], s[1], s[3] + s[5]))
            out_vb[0, :, ci * HS:(ci + 1) * HS, cj * WS:(cj + 1) * WS] = (
                np.ascontiguousarray(v).reshape(D * D, HS, WS))

    imgp = np.zeros((H + 2 * PAD, W + 2 * PAD), np.float32)
    imgp[PAD:PAD + H, PAD:PAD + W] = img[0, 0]
    si = imgp.strides
    iv = np.lib.stride_tricks.as_strided(
        imgp, shape=(D, D, H, W), strides=(si[0], si[1], si[0], si[1]))
    out_img = np.ascontiguousarray(iv).reshape(1, D * D, H, W)
    return out_vb, out_img


def kernel(x_1, x_2, img, p_1, p_2, _trace=False):
    nc = _get_nc()
    in_maps = _prep_in_maps(np.asarray(x_1), np.asarray(x_2), np.asarray(img),
                            np.asarray(p_1), np.asarray(p_2))
    res = run_bass_kernel_spmd(nc, in_maps, list(range(NCORES)), trace=_trace)
    out = _postprocess(res.results, np.asarray(img))
    if _trace:
        return out, res
    return out


# revision 11
# speedup vs baseline: 1.1382x; 1.1382x over previous
"""Trainium2 Bass kernel for the FlowNet-style correlation layer.

Problem (hardcoded):
  x_1, x_2, p_1, p_2: [1, 64, 96, 96] f32;  img: [1, 1, 96, 96] f32
  x1 = concat(x_1, p_1) -> [1,128,96,96];  x2 = pad(concat(x_2,p_2), 20)
  out_vb[d, h, w]  = sum_c x1[c,h,w] * x2[c, h+dy, w+dx],  d = dy*41+dx
  out_img[d, h, w] = pad(img,20)[h+dy, w+dx]
  returns (out_vb [1,1681,96,96], out_img [1,1681,96,96])

Strategy: 8 cores tile the output plane 4x2 (24 h-rows x 48 w-cols each).
Within a core, 9 groups of 8h x 16w x1 pixels are packed as the 128-wide
stationary operand (hi*16+wj), so every PE column is live. The moving
operand is a 48-row x 64-col window of the core's zero-padded x2 slab; the
TensorEngine contracts over the 128 concat-channels, producing the group
Gram G[(hi,wj), (t,v)] = sum_c x1[c,hi,wj] * x2pad[c, hi+t.., wj+v..] in
fp32 PSUM. The correlation band is G[., (hi+dy, wj+dx)] -- every (dy,dx)
displacement of every packed pixel lands inside the 48x56 useful window
(redundancy only 1.63x vs the 2.6x of a full-row Gram). A single bf16
pass suffices for the 2e-2 gate (inputs are unit-normal; err ~3e-3).
PSUM is evacuated by Vector/Scalar/Pool copies that cast f32->f16 and
drop the 8 pad columns that round the matmul free size up to 512 (one
PSUM bank); the host extracts the diagonal band from the fp16 Gram with a
strided view while unsharding. out_img is a pure gather of the (host-
held) input image and is materialized host-side.
"""

import numpy as np

import concourse.bass as bass
import concourse.tile as tile
from concourse import bacc, mybir
from concourse.bass_types import AP
from concourse.bass_utils import run_bass_kernel_spmd

F32 = mybir.dt.float32
F16 = mybir.dt.float16
BF16 = mybir.dt.bfloat16

H = W = 96
C2 = 128            # concat channels
PAD = 20
D = 2 * PAD + 1     # 41 displacements per axis
NCORES = 8
CH, CW = 4, 2       # core grid (h x w)
HS = H // CH        # 24 output rows per core
WS = W // CW        # 48 output cols per core
HG, WG = 8, 16      # stationary packing: 8 h-rows x 16 w-cols = 128
NGH, NGW = HS // HG, WS // WG   # 3 x 3 groups per core
NG = NGH * NGW
TR = HG + D - 1     # 48 gram rows (t = hi + dy)
TV = WG + D - 1     # 56 useful gram cols (v = wj + dx)
TVP = 64            # padded to 64 so each matmul is exactly one PSUM bank
SR = HS + 2 * PAD   # 64 x2 slab rows per core
SC = WS + 2 * PAD + (TVP - TV)  # 96 x2 slab cols (incl. 8 junk-read pad)
GSZ = TR * TV       # 2688 gram elems per partition per group


def _build_nc():
    nc = bacc.Bacc("TRN2", target_bir_lowering=False, debug=False,
                   num_devices=NCORES)

    x1p = nc.declare_dram_parameter("x1p", [C2, NG * 128], BF16,
                                    isOutput=False)
    x2s = nc.declare_dram_parameter("x2s", [C2, SR, SC], BF16, isOutput=False)
    # gram pairs two consecutive groups per partition-row so each store
    # moves 10752B DMA packets (per-queue dispatch is ~23ns/packet, so
    # 5376B packets cap a queue at ~230 GB/s). Pair 4 only fills gi=0.
    NPAIR = (NG + 1) // 2
    gram = nc.declare_dram_parameter("gram", [NPAIR, 128, 2, GSZ], F16,
                                     isOutput=True)

    with tile.TileContext(nc) as tc:
        with (
            tc.tile_pool(name="inp", bufs=1) as pin,
            tc.tile_pool(name="stage", bufs=3) as pst,
            tc.tile_pool(name="psum", bufs=4, space="PSUM") as pps,
        ):
            x1p_sb = pin.tile([C2, NG * 128], BF16)
            nc.sync.dma_start(x1p_sb[:], x1p[:])
            x2s_sb = pin.tile([C2, SR, SC], BF16)
            # rows [0, 48) feed the first (gh=0) groups; stream them first,
            # byte-balanced across both DMA queues (sync also carries x1p),
            # so matmuls start ASAP while rows [48, 64) trail in.
            nc.scalar.dma_start(x2s_sb[:, 0:26, :], x2s[:, 0:26, :])
            nc.sync.dma_start(x2s_sb[:, 26:TR, :], x2s[:, 26:TR, :])
            nc.scalar.dma_start(x2s_sb[:, TR:, :], x2s[:, TR:, :])

            def _copy(k, dst, src):
                if k % 2 == 0:
                    nc.vector.tensor_copy(dst, src)
                else:
                    nc.scalar.copy(dst, src)

            deferred = []

            def _store(pair, stage, npg):
                dst = AP(tensor=gram[:].tensor,
                         offset=pair * 128 * 2 * GSZ,
                         ap=[[2 * GSZ, 128], [1, npg * GSZ]])
                return dst, stage[:, 0:npg]

            for pair in range(NPAIR):
                # scalar's deferred store: by now its stage is long done,
                # so the dma_start enqueues without stalling scalar's
                # copy stream.
                if deferred:
                    nc.scalar.dma_start(*deferred.pop())
                gs = [g for g in (2 * pair, 2 * pair + 1) if g < NG]
                stage = pst.tile([C2, 2, 3, 2, HG, TV], F16)
                for gi, g in enumerate(gs):
                    gh, gw = divmod(g, NGW)
                    stat = x1p_sb[:, g * 128:(g + 1) * 128]
                    for third in range(3):
                        ps = pps.tile([C2, 2, HG, TVP], F32)  # 2 PSUM banks
                        for m in range(2):
                            r0 = HG * gh + HG * (third * 2 + m)
                            nc.tensor.matmul(
                                ps[:, m], stat,
                                x2s_sb[:, r0:r0 + HG, WG * gw:WG * gw + TVP],
                                start=True, stop=True)
                        _copy(g * 3 + third, stage[:, gi, third],
                              ps[:, :, :, 0:TV])
                if pair % 2 == 0:
                    nc.sync.dma_start(*_store(pair, stage, len(gs)))
                else:
                    deferred.append(_store(pair, stage, len(gs)))
            while deferred:
                nc.scalar.dma_start(*deferred.pop())

    nc.compile()
    return nc


_NC_CACHE = None


def _get_nc():
    global _NC_CACHE
    if _NC_CACHE is None:
        _NC_CACHE = _build_nc()
    return _NC_CACHE


def _prep_in_maps(x_1, x_2, img, p_1, p_2):
    import ml_dtypes
    bf = ml_dtypes.bfloat16

    x1f = np.concatenate([x_1[0], p_1[0]], axis=0).astype(bf)
    x2f = np.concatenate([x_2[0], p_2[0]], axis=0).astype(bf)
    x2pad = np.zeros((C2, H + 2 * PAD, W + 2 * PAD + (TVP - TV)), bf)
    x2pad[:, PAD:PAD + H, PAD:PAD + W] = x2f

    in_maps = []
    for ci in range(CH):
        for cj in range(CW):
            h0, w0 = ci * HS, cj * WS
            x1c = x1f[:, h0:h0 + HS, w0:w0 + WS]
            x1pk = (x1c.reshape(C2, NGH, HG, NGW, WG)
                    .transpose(0, 1, 3, 2, 4).reshape(C2, NG * 128))
            in_maps.append({
                "x1p": np.ascontiguousarray(x1pk),
                "x2s": np.ascontiguousarray(
                    x2pad[:, h0:h0 + SR, w0:w0 + SC]),
            })
    return in_maps


def _postprocess(results, img):
    out_vb = np.empty((1, D * D, H, W), np.float32)
    k = 0
    for ci in range(CH):
        for cj in range(CW):
            gp = np.asarray(results[k]["gram"])   # [5, 128, 2, 2688] f16
            k += 1
            g16 = gp.transpose(0, 2, 1, 3).reshape(-1, 128, GSZ)[:NG]
            A = g16.reshape(NGH, NGW, HG, WG, TR, TV)
            s = A.strides
            v = np.lib.stride_tricks.as_strided(
                A, shape=(D, D, NGH, HG, NGW, WG),
                strides=(s[4], s[5], s[0], s[2] + s[4], s[1], s[3] + s[5]))
            out_vb[0, :, ci * HS:(ci + 1) * HS, cj * WS:(cj + 1) * WS] = (
                np.ascontiguousarray(v).reshape(D * D, HS, WS))

    imgp = np.zeros((H + 2 * PAD, W + 2 * PAD), np.float32)
    imgp[PAD:PAD + H, PAD:PAD + W] = img[0, 0]
    si = imgp.strides
    iv = np.lib.stride_tricks.as_strided(
        imgp, shape=(D, D, H, W), strides=(si[0], si[1], si[0], si[1]))
    out_img = np.ascontiguousarray(iv).reshape(1, D * D, H, W)
    return out_vb, out_img


def kernel(x_1, x_2, img, p_1, p_2, _trace=False):
    nc = _get_nc()
    in_maps = _prep_in_maps(np.asarray(x_1), np.asarray(x_2), np.asarray(img),
                            np.asarray(p_1), np.asarray(p_2))
    res = run_bass_kernel_spmd(nc, in_maps, list(range(NCORES)), trace=_trace)
    out = _postprocess(res.results, np.asarray(img))
    if _trace:
        return out, res
    return out
